# revision 4
# baseline (speedup 1.0000x reference)
"""Causal self-attention (B=4, S=2048, D=1024, single head) on 8 TRN2 cores.

Sharding: data-parallel over batch (4 batches x 2 cores), with the two cores
of a batch splitting the KEY dimension (core par=0 takes even key tiles,
par=1 odd). Each core projects Q for ALL 2048 queries of its batch but K/V
only for its 8 key tiles, computes scores TRANSPOSED (S^T tiles [key, query]
directly off PE — no PE transposes needed), exponentiates into a packed P^T
buffer, and accumulates an UNNORMALIZED numerator N = P^T.T @ V plus the
denominator l = colsum(P^T). The host combines the two partials per batch:
out = (N_even + N_odd) / (l_even + l_odd).

vs the query-split baseline this trades the duplicated K+V projection
(2x 4.3 GFLOP/core) for a duplicated Q projection (2.1 GFLOP/core) and
removes all 72 PE transposes per core.

Per key tile jl (global tile j = 2*jl+par), scores are computed against the
query range [2*jl*128, 2048) — one program for both cores; the first 256
query columns carry a host-built additive mask (triangular diagonal for the
variant where j is the range start, full -60 + triangular for the other).
"""

import os
from contextlib import ExitStack

import ml_dtypes
import numpy as np

import concourse.bacc as bacc
import concourse.mybir as mybir
import concourse.tile as tile
from concourse.bass_utils import run_bass_kernel_spmd

B, S, D = 4, 2048, 1024
P = 128
DC = D // P  # 8 contraction chunks
NKT = 8      # own key tiles per core
MASK_VAL = -60.0

F32 = mybir.dt.float32
F16 = mybir.dt.float16
BF16 = mybir.dt.bfloat16

W_JL = [(16 - 2 * jl) * P for jl in range(NKT)]  # 2048,1792,...,256
OFF_JL = [0]
for _w in W_JL[:-1]:
    OFF_JL.append(OFF_JL[-1] + _w)
PT_COLS = OFF_JL[-1] + W_JL[-1]  # 9216

_compiled = {}


def _build():
    nc = bacc.Bacc("TRN2", target_bir_lowering=False, debug=False)
    xT = nc.dram_tensor("xT", [D, S], BF16, kind="ExternalInput").ap()       # x^T/32
    xkT = nc.dram_tensor("xkT", [D, NKT * P], BF16, kind="ExternalInput").ap()
    wq = nc.dram_tensor("wq", [D, D], BF16, kind="ExternalInput").ap()
    wk = nc.dram_tensor("wk", [D, D], BF16, kind="ExternalInput").ap()
    wv = nc.dram_tensor("wv", [D, D], BF16, kind="ExternalInput").ap()
    msk = nc.dram_tensor("msk", [NKT * P, 2 * P], F32, kind="ExternalInput").ap()
    out_d = nc.dram_tensor("out", [S, D], F16, kind="ExternalOutput").ap()
    lout = nc.dram_tensor("lout", [1, S], F32, kind="ExternalOutput").ap()
    with tile.TileContext(nc) as tc:
        _body(tc, xT, xkT, wq, wk, wv, msk, out_d, lout)
    nc.compile()
    return nc


def _body(tc, xT, xkT, wq, wk, wv, msk, out_d, lout):
    nc = tc.nc
    with ExitStack() as top:
        const_pool = top.enter_context(tc.tile_pool(name="cst", bufs=1))
        ones = const_pool.tile([P, 1], BF16, name="ones", tag="ones")
        nc.gpsimd.memset(ones[:], 1.0)

        res = top.enter_context(tc.tile_pool(name="res", bufs=1))
        QT = [res.tile([P, S], BF16, name=f"qt{e}", tag=f"qt{e}") for e in range(DC)]
        KT = [res.tile([P, NKT * P], BF16, name=f"kt{e}", tag=f"kt{e}") for e in range(DC)]
        V = [res.tile([P, D], BF16, name=f"v{j}", tag=f"v{j}") for j in range(NKT)]
        PT = res.tile([P, PT_COLS], BF16, name="ptb", tag="ptb")
        lsb = res.tile([1, S], F32, name="lsb", tag="lsb")

        psum = top.enter_context(tc.tile_pool(name="psum", bufs=2, space="PSUM"))

        # ---------------- projections: V (first — smallest DMA gate), K, Q ----
        with ExitStack() as ph:
            wvp = ph.enter_context(tc.tile_pool(name="wvp", bufs=1))
            wkp = ph.enter_context(tc.tile_pool(name="wkp", bufs=1))
            wqp = ph.enter_context(tc.tile_pool(name="wqp", bufs=1))
            xkp = ph.enter_context(tc.tile_pool(name="xkp", bufs=1))
            xqp = ph.enter_context(tc.tile_pool(name="xqp", bufs=1))
            wv_t = [wvp.tile([P, D], BF16, name=f"wv{d}", tag=f"wv{d}") for d in range(DC)]
            wk_t = [wkp.tile([P, D], BF16, name=f"wk{d}", tag=f"wk{d}") for d in range(DC)]
            wq_t = [wqp.tile([P, D], BF16, name=f"wq{d}", tag=f"wq{d}") for d in range(DC)]
            xk_t = [xkp.tile([P, NKT * P], BF16, name=f"xk{d}", tag=f"xk{d}") for d in range(DC)]
            xq_t = [xqp.tile([P, S], BF16, name=f"xq{d}", tag=f"xq{d}") for d in range(DC)]

            # DMA issue order = dependency order of the compute below.
            for d in range(DC):
                nc.sync.dma_start(xk_t[d][:, 0:512], xkT[d * P : (d + 1) * P, 0:512])
            for d in range(DC):
                nc.sync.dma_start(wv_t[d][:, 0:512], wv[d * P : (d + 1) * P, 0:512])
            for d in range(DC):
                nc.sync.dma_start(xk_t[d][:, 512:1024], xkT[d * P : (d + 1) * P, 512:1024])
            for d in range(DC):
                nc.sync.dma_start(wv_t[d][:, 512:1024], wv[d * P : (d + 1) * P, 512:1024])
            for d in range(DC):
                nc.sync.dma_start(wk_t[d][:], wk[d * P : (d + 1) * P, :])
            for d in range(DC):
                nc.sync.dma_start(wq_t[d][:], wq[d * P : (d + 1) * P, :])
            for d in range(DC):
                for qc in range(4):
                    nc.sync.dma_start(
                        xq_t[d][:, qc * 512 : (qc + 1) * 512],
                        xT[d * P : (d + 1) * P, qc * 512 : (qc + 1) * 512],
                    )

            # V[jl] = x_j @ Wv  (x stationary, Wv moving)
            for jl in range(NKT):
                for ec in range(2):
                    ps = psum.tile([P, 512], F32, name="pps", tag="pps")
                    for d in range(DC):
                        nc.tensor.matmul(
                            ps[:],
                            lhsT=xk_t[d][:, jl * P : (jl + 1) * P],
                            rhs=wv_t[d][:, ec * 512 : (ec + 1) * 512],
                            start=(d == 0),
                            stop=(d == DC - 1),
                        )
                    nc.scalar.copy(V[jl][:, ec * 512 : (ec + 1) * 512], ps[:])

            # K^T (e-major over own keys)
            for e in range(DC):
                for kc in range(2):
                    ps = psum.tile([P, 512], F32, name="pps", tag="pps")
                    for d in range(DC):
                        nc.tensor.matmul(
                            ps[:],
                            lhsT=wk_t[d][:, e * P : (e + 1) * P],
                            rhs=xk_t[d][:, kc * 512 : (kc + 1) * 512],
                            start=(d == 0),
                            stop=(d == DC - 1),
                        )
                    nc.scalar.copy(KT[e][:, kc * 512 : (kc + 1) * 512], ps[:])

            # Q^T (e-major over ALL queries; x pre-scaled by 1/32)
            for e in range(DC):
                for qc in range(4):
                    ps = psum.tile([P, 512], F32, name="pps", tag="pps")
                    for d in range(DC):
                        nc.tensor.matmul(
                            ps[:],
                            lhsT=wq_t[d][:, e * P : (e + 1) * P],
                            rhs=xq_t[d][:, qc * 512 : (qc + 1) * 512],
                            start=(d == 0),
                            stop=(d == DC - 1),
                        )
                    nc.scalar.copy(QT[e][:, qc * 512 : (qc + 1) * 512], ps[:])

        # ---------------- attention ----------------
        with ExitStack() as ph:
            mp = ph.enter_context(tc.tile_pool(name="mp", bufs=2))
            ob = ph.enter_context(tc.tile_pool(name="ob", bufs=3))

            # scores^T + exp, per own key tile
            for jl in range(NKT):
                Wj = W_JL[jl]
                qb = 2 * jl * P
                nch = (Wj + 511) // 512
                mt = mp.tile([P, 2 * P], F32, name="mt", tag="mt")
                nc.sync.dma_start(mt[:], msk[jl * P : (jl + 1) * P, :])
                for c in range(nch):
                    nw = min(512, Wj - c * 512)
                    ps = psum.tile([P, 512], F32, name="qk", tag="qk")
                    for e in range(DC):
                        nc.tensor.matmul(
                            ps[:, :nw],
                            lhsT=KT[e][:, jl * P : (jl + 1) * P],
                            rhs=QT[e][:, qb + c * 512 : qb + c * 512 + nw],
                            start=(e == 0),
                            stop=(e == DC - 1),
                        )
                    if c == 0:
                        nc.vector.tensor_add(ps[:, 0 : 2 * P], ps[:, 0 : 2 * P], mt[:])
                    nc.scalar.activation(
                        PT[:, OFF_JL[jl] + c * 512 : OFF_JL[jl] + c * 512 + nw],
                        ps[:, :nw],
                        mybir.ActivationFunctionType.Exp,
                    )

            # denominators: l[q] = sum_k P^T[k, q] via ones-stationary matmuls
            with ExitStack() as lh:
                lp = lh.enter_context(tc.tile_pool(name="lp", bufs=2, space="PSUM"))
                for qc in range(4):
                    q0, q1 = qc * 512, qc * 512 + 512
                    js = [jl for jl in range(NKT) if 2 * jl * P < q1]
                    lps = lp.tile([1, 512], F32, name="lps", tag="lps")
                    for i, jl in enumerate(js):
                        s = max(q0, 2 * jl * P)
                        w = q1 - s
                        o = OFF_JL[jl] + (s - 2 * jl * P)
                        nc.tensor.matmul(
                            lps[0:1, s - q0 : 512],
                            lhsT=ones[:],
                            rhs=PT[:, o : o + w],
                            start=(i == 0),
                            stop=(i == len(js) - 1),
                            skip_group_check=True,
                        )
                    nc.scalar.copy(lsb[0:1, q0:q1], lps[:])
                nc.sync.dma_start(lout[0:1, :], lsb[:])

            # numerators: N[t] = sum_jl P_tile^T.T @ V[jl]; descending t for a
            # short tail (t=0 is a single-matmul group).
            for t in reversed(range(16)):
                njl = t // 2 + 1
                for ec in range(2):
                    ops = psum.tile([P, 512], F32, name="ops", tag="ops")
                    for jl in range(njl):
                        o = OFF_JL[jl] + (t - 2 * jl) * P
                        nc.tensor.matmul(
                            ops[:],
                            lhsT=PT[:, o : o + P],
                            rhs=V[jl][:, ec * 512 : (ec + 1) * 512],
                            start=(jl == 0),
                            stop=(jl == njl - 1),
                        )
                    ot = ob.tile([P, 512], F16, name="ot", tag="ot")
                    if ec == 0:
                        nc.scalar.copy(ot[:], ops[:])
                    else:
                        nc.vector.tensor_copy(ot[:], ops[:])
                    nc.sync.dma_start(
                        out_d[t * P : (t + 1) * P, ec * 512 : (ec + 1) * 512], ot[:]
                    )


def _get_nc():
    if "nc" not in _compiled:
        _compiled["nc"] = _build()
    return _compiled["nc"]


def kernel(x, Wq, Wk, Wv):
    x = np.ascontiguousarray(np.asarray(x, dtype=np.float32))
    Wq = np.asarray(Wq, dtype=np.float32)
    Wk = np.asarray(Wk, dtype=np.float32)
    Wv = np.asarray(Wv, dtype=np.float32)

    nc = _get_nc()
    bf16 = ml_dtypes.bfloat16
    Wq_c = np.ascontiguousarray(Wq.astype(bf16))
    Wk_c = np.ascontiguousarray(Wk.astype(bf16))
    Wv_c = np.ascontiguousarray(Wv.astype(bf16))

    in_maps = []
    for c in range(8):
        b, par = c // 2, c % 2
        xb = x[b]  # [S, D]
        xT_np = np.ascontiguousarray((xb.T * np.float32(1.0 / 32.0)).astype(bf16))
        keys = np.concatenate(
            [np.arange((2 * i + par) * P, (2 * i + par + 1) * P) for i in range(NKT)]
        )
        xkT_np = np.ascontiguousarray(xb.T[:, keys].astype(bf16))
        m = np.empty((NKT * P, 2 * P), np.float32)
        for jl in range(NKT):
            j = 2 * jl + par
            kglob = np.arange(j * P, (j + 1) * P)
            qglob = np.arange(2 * jl * P, 2 * jl * P + 2 * P)
            m[jl * P : (jl + 1) * P, :] = np.where(
                qglob[None, :] >= kglob[:, None], np.float32(0.0), np.float32(MASK_VAL)
            )
        in_maps.append(
            {
                "xT": xT_np,
                "xkT": xkT_np,
                "wq": Wq_c,
                "wk": Wk_c,
                "wv": Wv_c,
                "msk": np.ascontiguousarray(m),
            }
        )

    trace = os.environ.get("BASS_KERNEL_TRACE", "0") == "1"
    res = run_bass_kernel_spmd(nc, in_maps, core_ids=list(range(8)), trace=trace)
    if trace:
        print(f"HW exec time: {res.exec_time_ns} ns")
        if res.instructions_and_trace is not None:
            print(f"trace: {res.instructions_and_trace[1]}")

    out = np.empty((B, S, D), np.float32)
    for b in range(B):
        n0 = res.results[2 * b]["out"].astype(np.float32)
        n1 = res.results[2 * b + 1]["out"].astype(np.float32)
        l0 = res.results[2 * b]["lout"][0].astype(np.float32)
        l1 = res.results[2 * b + 1]["lout"][0].astype(np.float32)
        out[b] = (n0 + n1) / (l0 + l1)[:, None]
    return out
